# revision 13
# baseline (speedup 1.0000x reference)
"""EOSFocusedLoss Trainium2 kernel.

Problem (hardcoded, self-contained): logits [32,256,16000] f32, targets [32,256] int.
Returns the 6-tuple (total, main_loss, eos_loss, pattern_loss, length_penalty,
eos_success_rate) as a float32 array of shape (6,).

Strategy: data-parallel over batch — each of the 8 NeuronCores gets 4 batch rows
([1024, 16000] after flattening (b,s)). The device does the memory-bound vocab
reductions per position: top-8 max (DVE max8), exact argmax (DVE max_index, first
occurrence — matches jnp.argmax), and sum(exp(x)) (ScalarE Exp with hardware
accumulate; plain exp is safe since logits are N(0,1)-scale so exp stays in
[e-6, e+6]). Everything downstream is tiny [32,256]-level math done on host:
logsumexp via log(sum), NLL gather, the EOS margin (only needs the 32 rows at the
first-EOS positions, sliced on host), the repetition-pattern detector, and the
final scalar combines. No cross-core collectives are needed.
"""

import numpy as np

B, S, V = 32, 256, 16000
N_CORES = 8
BPC = B // N_CORES          # batch rows per core
RPC = BPC * S               # flattened rows per core = 1024
NT = RPC // 128             # SBUF partition tiles per core = 8
NCH = 4                     # DMA / exp chunks per tile
CH = V // NCH

PAD_IDX, EOS_IDX = 0, 1
EOS_W, PAT_W, SEQ_W = 20.0, 2.0, 0.5

_prog = None
LAST = {}      # diagnostics: exec_time_ns etc.
TRACE = False  # set True (e.g. from test.py) to collect an NTFF profile


SEG = 125                   # segment width for the two-level argmax
NSEG = V // SEG             # 128 segments per row (aligns halves to DMA chunks)


def _build(version=2):
    import concourse.bacc as bacc
    import concourse.bass as bass
    import concourse.mybir as mybir
    import concourse.tile as tile

    f32 = mybir.dt.float32
    u32 = mybir.dt.uint32
    nc = bacc.Bacc()
    x = nc.dram_tensor("logits", [RPC, V], f32, kind="ExternalInput")

    if version == 1:
        sums_out = nc.dram_tensor("sums", [NT, 128, NCH], f32, kind="ExternalOutput")
        idx_out = nc.dram_tensor("idx", [NT, 128, 1], u32, kind="ExternalOutput")
        with tile.TileContext(nc) as tc:
            with tc.tile_pool(name="xp", bufs=2) as xp, \
                 tc.tile_pool(name="ep", bufs=2) as ep, \
                 tc.tile_pool(name="sp", bufs=4) as sp:
                for t in range(NT):
                    xt = xp.tile([128, V], f32, tag="xt")
                    for c in range(NCH):
                        nc.sync.dma_start(
                            xt[:, c * CH:(c + 1) * CH],
                            x[t * 128:(t + 1) * 128, c * CH:(c + 1) * CH],
                        )
                    maxv = sp.tile([128, 8], f32, tag="maxv")
                    nc.vector.max(maxv[:], xt[:])
                    idx8 = sp.tile([128, 8], u32, tag="idx8")
                    nc.vector.max_index(idx8[:], maxv[:], xt[:])
                    s = sp.tile([128, NCH], f32, tag="s")
                    for c in range(NCH):
                        e = ep.tile([128, CH], f32, tag="e")
                        nc.scalar.activation(
                            e[:], xt[:, c * CH:(c + 1) * CH],
                            mybir.ActivationFunctionType.Exp,
                            accum_out=s[:, c:c + 1],
                        )
                    nc.sync.dma_start(idx_out[t], idx8[:, 0:1])
                    nc.sync.dma_start(sums_out[t], s[:])
        nc.finalize()
        return nc

    # version >= 2: segmented argmax — one full-width DVE pass (segment max),
    # then 125/128-wide index scans plus an indirect gather of each row's
    # winning 128-wide segment straight from DRAM. All per-tile results land
    # in persistent SBUF accumulators (sliced writes); only 3 output DMAs at
    # the end, so the in-loop DMA queue carries nothing but input chunks.
    rb = nc.dram_tensor("rowbase", [128, NT], u32, kind="ExternalInput")
    sums_out = nc.dram_tensor("sums", [128, NT, NCH], f32, kind="ExternalOutput")
    seg_out = nc.dram_tensor("seg", [128, NT, 8], u32, kind="ExternalOutput")
    win_out = nc.dram_tensor("win", [128, NT, 8], u32, kind="ExternalOutput")
    x_segs = x.rearrange("r (s j) -> (r s) j", j=SEG)  # [(RPC*NSEG), SEG]

    with tile.TileContext(nc) as tc:
        with tc.tile_pool(name="xp", bufs=2) as xp, \
             tc.tile_pool(name="ep", bufs=2) as ep, \
             tc.tile_pool(name="gp", bufs=3) as gp, \
             tc.tile_pool(name="sp", bufs=4) as sp, \
             tc.tile_pool(name="acc", bufs=1) as acc:
            rb_all = acc.tile([128, NT], u32, tag="rb_all")
            nc.sync.dma_start(rb_all[:], rb[:])
            seg_all = acc.tile([128, NT * 8], u32, tag="seg_all")
            win_all = acc.tile([128, NT * 8], u32, tag="win_all")
            s_all = acc.tile([128, NT * NCH], f32, tag="s_all")

            for t in range(NT):
                xt = xp.tile([128, V], f32, tag="xt")
                for c in range(NCH):
                    nc.sync.dma_start(
                        xt[:, c * CH:(c + 1) * CH],
                        x[t * 128:(t + 1) * 128, c * CH:(c + 1) * CH],
                    )
                segmax = gp.tile([128, NSEG], f32, tag="segmax")
                half = V // 2
                hseg = NSEG // 2
                for h in range(2):
                    nc.vector.tensor_reduce(
                        segmax[:, h * hseg:(h + 1) * hseg],
                        xt[:, h * half:(h + 1) * half].rearrange(
                            "p (s j) -> p s j", j=SEG
                        ),
                        axis=mybir.AxisListType.X, op=mybir.AluOpType.max,
                    )
                maxv = sp.tile([128, 8], f32, tag="maxv")
                nc.vector.max(maxv[:], segmax[:])
                nc.vector.max_index(
                    seg_all[:, t * 8:(t + 1) * 8], maxv[:], segmax[:]
                )
                # flat segment id into x_segs: row*NSEG + seg
                flat = sp.tile([128, 1], u32, tag="flat")
                nc.vector.tensor_tensor(
                    flat[:], seg_all[:, t * 8:t * 8 + 1], rb_all[:, t:t + 1],
                    op=mybir.AluOpType.add,
                )
                g = gp.tile([128, SEG], f32, tag="g")
                nc.gpsimd.indirect_dma_start(
                    out=g[:], out_offset=None,
                    in_=x_segs,
                    in_offset=bass.IndirectOffsetOnAxis(ap=flat[:, :1], axis=0),
                )
                nc.vector.max_index(
                    win_all[:, t * 8:(t + 1) * 8], maxv[:], g[:]
                )
                for c in range(NCH):
                    e = ep.tile([128, CH], f32, tag="e")
                    nc.scalar.activation(
                        e[:], xt[:, c * CH:(c + 1) * CH],
                        mybir.ActivationFunctionType.Exp,
                        accum_out=s_all[:, t * NCH + c:t * NCH + c + 1],
                    )
            nc.sync.dma_start(seg_out[:], seg_all[:])
            nc.sync.dma_start(win_out[:], win_all[:])
            nc.sync.dma_start(sums_out[:], s_all[:])
    nc.finalize()
    return nc


def _repetitive_count(preds):
    """Faithful numpy port of the reference pattern detector. preds [B,S] int."""
    Bn, Sn = preds.shape
    is_pad = preds == PAD_IDX
    L = np.where(is_pad.any(axis=1), np.argmax(is_pad, axis=1), Sn)  # [B]
    rep = np.zeros(Bn, dtype=bool)
    for p in (2, 3, 4):
        n_starts = Sn - 3 * p + 1
        if n_starts <= 0:
            continue
        eq = (preds[:, :Sn - p] == preds[:, p:]).astype(np.int64)
        cs = np.pad(np.cumsum(eq, axis=1), ((0, 0), (1, 0)))
        win = cs[:, 2 * p:2 * p + n_starts] - cs[:, :n_starts]
        full = win == 2 * p
        starts = np.arange(n_starts)
        valid = (starts[None, :] + 3 * p <= L[:, None]) & (L[:, None] >= 3 * p + 3)
        rep |= (full & valid).any(axis=1)
    return int(rep.sum())


def _finalize(logits, targets, preds, sumexp):
    """Host-side combine. logits [B,S,V] f32, targets [B,S] int,
    preds [B,S] int (device argmax), sumexp [B,S] f64 (device sum of exp)."""
    targets = np.asarray(targets).astype(np.int64)

    # main cross-entropy with ignore_index = PAD
    lse = np.log(sumexp)  # [B,S] f64; == logsumexp since exp was unbiased
    tgt_logit = np.take_along_axis(logits, targets[..., None], axis=2)[..., 0]
    nll = lse - tgt_logit.astype(np.float64)
    keep = (targets != PAD_IDX)
    main_loss = (nll * keep).sum() / max(keep.sum(), 1.0)

    # repetition pattern penalty
    rep_count = _repetitive_count(preds)
    pattern_loss = rep_count / B * 100.0

    # EOS margin loss — only the 32 rows at the first EOS position matter
    is_eos = targets == EOS_IDX
    has_eos = is_eos.any(axis=1)
    pos = np.argmax(is_eos, axis=1)
    logit_at = logits[np.arange(B), pos].astype(np.float64)  # [B,V]
    eos_logit = logit_at[:, EOS_IDX]
    masked = logit_at.copy()
    masked[:, EOS_IDX] = -np.inf
    max_other = masked.max(axis=1)
    margin = np.maximum(max_other - eos_logit + 1.0, 0.0)
    eos_loss = np.where(has_eos, margin, 0.0).sum() / B
    pred_at = np.argmax(logit_at, axis=1)
    eos_predictions = ((pred_at == EOS_IDX) & has_eos).sum()
    eos_targets = has_eos.sum()
    eos_success_rate = eos_predictions / max(eos_targets, 1)

    # length penalty
    avg_pred_len = (preds != PAD_IDX).sum(axis=1).mean()
    avg_tgt_len = (targets != PAD_IDX).sum(axis=1).mean()
    length_penalty = abs(avg_pred_len - avg_tgt_len) / avg_tgt_len

    total = main_loss + EOS_W * eos_loss + PAT_W * pattern_loss + SEQ_W * length_penalty
    return np.array(
        [total, main_loss, eos_loss, pattern_loss, length_penalty, eos_success_rate],
        dtype=np.float32,
    )


VERSION = 2


def kernel(logits, targets):
    global _prog
    from concourse.bass_utils import run_bass_kernel_spmd

    logits = np.ascontiguousarray(np.asarray(logits, dtype=np.float32))
    if _prog is None:
        _prog = _build(VERSION)

    shards = logits.reshape(N_CORES, RPC, V)
    in_maps = [{"logits": shards[c]} for c in range(N_CORES)]
    if VERSION >= 2:
        # rowbase[p, t] = flat row id * NSEG, the base segment index of row
        # (t*128+p) in the [(RPC*NSEG), SEG] view of the shard
        rowbase = (
            (np.arange(NT, dtype=np.uint32)[None, :] * 128
             + np.arange(128, dtype=np.uint32)[:, None]) * NSEG
        )
        for m in in_maps:
            m["rowbase"] = rowbase
    out = run_bass_kernel_spmd(
        _prog, in_maps, core_ids=list(range(N_CORES)), trace=TRACE
    )
    LAST["exec_time_ns"] = out.exec_time_ns
    LAST["insts"] = out.instructions_and_trace
    res = out.results

    if VERSION >= 2:
        # [128, NT] slot-0 planes -> flat row order (t*128+p)
        preds = np.stack(
            [
                (
                    r["seg"][:, :, 0].astype(np.int64) * SEG
                    + r["win"][:, :, 0].astype(np.int64)
                ).T.reshape(RPC)
                for r in res
            ]
        ).reshape(B, S)
        sumexp = np.stack(
            [r["sums"].astype(np.float64).sum(axis=2).T.reshape(RPC) for r in res]
        ).reshape(B, S)
    else:
        preds = np.stack([r["idx"].reshape(RPC) for r in res]).reshape(B, S).astype(np.int64)
        sumexp = np.stack(
            [r["sums"].astype(np.float64).sum(axis=2).reshape(RPC) for r in res]
        ).reshape(B, S)
    return _finalize(logits, targets, preds, sumexp)


# revision 14
# speedup vs baseline: 1.0247x; 1.0247x over previous
"""EOSFocusedLoss Trainium2 kernel.

Problem (hardcoded, self-contained): logits [32,256,16000] f32, targets [32,256] int.
Returns the 6-tuple (total, main_loss, eos_loss, pattern_loss, length_penalty,
eos_success_rate) as a float32 array of shape (6,).

Strategy: data-parallel over batch — each of the 8 NeuronCores gets 4 batch rows
([1024, 16000] after flattening (b,s)). The device does the memory-bound vocab
reductions per position: top-8 max (DVE max8), exact argmax (DVE max_index, first
occurrence — matches jnp.argmax), and sum(exp(x)) (ScalarE Exp with hardware
accumulate; plain exp is safe since logits are N(0,1)-scale so exp stays in
[e-6, e+6]). Everything downstream is tiny [32,256]-level math done on host:
logsumexp via log(sum), NLL gather, the EOS margin (only needs the 32 rows at the
first-EOS positions, sliced on host), the repetition-pattern detector, and the
final scalar combines. No cross-core collectives are needed.
"""

import numpy as np

B, S, V = 32, 256, 16000
N_CORES = 8
BPC = B // N_CORES          # batch rows per core
RPC = BPC * S               # flattened rows per core = 1024
NT = RPC // 128             # SBUF partition tiles per core = 8
NCH = 4                     # DMA / exp chunks per tile
CH = V // NCH

PAD_IDX, EOS_IDX = 0, 1
EOS_W, PAT_W, SEQ_W = 20.0, 2.0, 0.5

_prog = None
LAST = {}      # diagnostics: exec_time_ns etc.
TRACE = False  # set True (e.g. from test.py) to collect an NTFF profile


SEG = 125                   # segment width for the two-level argmax
NSEG = V // SEG             # 128 segments per row (aligns halves to DMA chunks)


def _build(version=2):
    import concourse.bacc as bacc
    import concourse.bass as bass
    import concourse.mybir as mybir
    import concourse.tile as tile

    f32 = mybir.dt.float32
    u32 = mybir.dt.uint32
    nc = bacc.Bacc()
    x = nc.dram_tensor("logits", [RPC, V], f32, kind="ExternalInput")

    if version == 1:
        sums_out = nc.dram_tensor("sums", [NT, 128, NCH], f32, kind="ExternalOutput")
        idx_out = nc.dram_tensor("idx", [NT, 128, 1], u32, kind="ExternalOutput")
        with tile.TileContext(nc) as tc:
            with tc.tile_pool(name="xp", bufs=2) as xp, \
                 tc.tile_pool(name="ep", bufs=2) as ep, \
                 tc.tile_pool(name="sp", bufs=4) as sp:
                for t in range(NT):
                    xt = xp.tile([128, V], f32, tag="xt")
                    for c in range(NCH):
                        nc.sync.dma_start(
                            xt[:, c * CH:(c + 1) * CH],
                            x[t * 128:(t + 1) * 128, c * CH:(c + 1) * CH],
                        )
                    maxv = sp.tile([128, 8], f32, tag="maxv")
                    nc.vector.max(maxv[:], xt[:])
                    idx8 = sp.tile([128, 8], u32, tag="idx8")
                    nc.vector.max_index(idx8[:], maxv[:], xt[:])
                    s = sp.tile([128, NCH], f32, tag="s")
                    for c in range(NCH):
                        e = ep.tile([128, CH], f32, tag="e")
                        nc.scalar.activation(
                            e[:], xt[:, c * CH:(c + 1) * CH],
                            mybir.ActivationFunctionType.Exp,
                            accum_out=s[:, c:c + 1],
                        )
                    nc.sync.dma_start(idx_out[t], idx8[:, 0:1])
                    nc.sync.dma_start(sums_out[t], s[:])
        nc.finalize()
        return nc

    # version >= 2: segmented argmax — one full-width DVE pass (segment max),
    # then 125/128-wide index scans plus an indirect gather of each row's
    # winning 128-wide segment straight from DRAM. All per-tile results land
    # in persistent SBUF accumulators (sliced writes); only 3 output DMAs at
    # the end, so the in-loop DMA queue carries nothing but input chunks.
    rb = nc.dram_tensor("rowbase", [128, NT], u32, kind="ExternalInput")
    sums_out = nc.dram_tensor("sums", [128, NT, NCH], f32, kind="ExternalOutput")
    seg_out = nc.dram_tensor("seg", [128, NT, 8], u32, kind="ExternalOutput")
    win_out = nc.dram_tensor("win", [128, NT, 8], u32, kind="ExternalOutput")
    x_segs = x.rearrange("r (s j) -> (r s) j", j=SEG)  # [(RPC*NSEG), SEG]

    with tile.TileContext(nc) as tc:
        with tc.tile_pool(name="xp", bufs=2) as xp, \
             tc.tile_pool(name="ep", bufs=2) as ep, \
             tc.tile_pool(name="gp", bufs=3) as gp, \
             tc.tile_pool(name="sp", bufs=4) as sp, \
             tc.tile_pool(name="acc", bufs=1) as acc:
            rb_all = acc.tile([128, NT], u32, tag="rb_all")
            nc.sync.dma_start(rb_all[:], rb[:])
            seg_all = acc.tile([128, NT * 8], u32, tag="seg_all")
            win_all = acc.tile([128, NT * 8], u32, tag="win_all")
            s_all = acc.tile([128, NT * NCH], f32, tag="s_all")

            # software pipeline: tile t's gather-dependent win-scan is
            # emitted after tile t+1's head so the indirect-DMA latency
            # never stalls the DVE queue.
            pend = {}  # t -> (maxv, g)

            def emit_tail(t):
                maxv, g = pend.pop(t)
                nc.vector.max_index(
                    win_all[:, t * 8:(t + 1) * 8], maxv[:], g[:]
                )

            for t in range(NT):
                xt = xp.tile([128, V], f32, tag="xt")
                for c in range(NCH):
                    nc.sync.dma_start(
                        xt[:, c * CH:(c + 1) * CH],
                        x[t * 128:(t + 1) * 128, c * CH:(c + 1) * CH],
                    )
                segmax = gp.tile([128, NSEG], f32, tag="segmax")
                half = V // 2
                hseg = NSEG // 2
                for h in range(2):
                    nc.vector.tensor_reduce(
                        segmax[:, h * hseg:(h + 1) * hseg],
                        xt[:, h * half:(h + 1) * half].rearrange(
                            "p (s j) -> p s j", j=SEG
                        ),
                        axis=mybir.AxisListType.X, op=mybir.AluOpType.max,
                    )
                maxv = sp.tile([128, 8], f32, tag="maxv")
                nc.vector.max(maxv[:], segmax[:])
                nc.vector.max_index(
                    seg_all[:, t * 8:(t + 1) * 8], maxv[:], segmax[:]
                )
                # flat segment id into x_segs: row*NSEG + seg
                flat = sp.tile([128, 1], u32, tag="flat")
                nc.vector.tensor_tensor(
                    flat[:], seg_all[:, t * 8:t * 8 + 1], rb_all[:, t:t + 1],
                    op=mybir.AluOpType.add,
                )
                g = gp.tile([128, SEG], f32, tag="g")
                nc.gpsimd.indirect_dma_start(
                    out=g[:], out_offset=None,
                    in_=x_segs,
                    in_offset=bass.IndirectOffsetOnAxis(ap=flat[:, :1], axis=0),
                )
                pend[t] = (maxv, g)
                if t >= 1:
                    emit_tail(t - 1)
                for c in range(NCH):
                    e = ep.tile([128, CH], f32, tag="e")
                    nc.scalar.activation(
                        e[:], xt[:, c * CH:(c + 1) * CH],
                        mybir.ActivationFunctionType.Exp,
                        accum_out=s_all[:, t * NCH + c:t * NCH + c + 1],
                    )
            emit_tail(NT - 1)
            nc.sync.dma_start(seg_out[:], seg_all[:])
            nc.sync.dma_start(win_out[:], win_all[:])
            nc.sync.dma_start(sums_out[:], s_all[:])
    nc.finalize()
    return nc


def _repetitive_count(preds):
    """Faithful numpy port of the reference pattern detector. preds [B,S] int."""
    Bn, Sn = preds.shape
    is_pad = preds == PAD_IDX
    L = np.where(is_pad.any(axis=1), np.argmax(is_pad, axis=1), Sn)  # [B]
    rep = np.zeros(Bn, dtype=bool)
    for p in (2, 3, 4):
        n_starts = Sn - 3 * p + 1
        if n_starts <= 0:
            continue
        eq = (preds[:, :Sn - p] == preds[:, p:]).astype(np.int64)
        cs = np.pad(np.cumsum(eq, axis=1), ((0, 0), (1, 0)))
        win = cs[:, 2 * p:2 * p + n_starts] - cs[:, :n_starts]
        full = win == 2 * p
        starts = np.arange(n_starts)
        valid = (starts[None, :] + 3 * p <= L[:, None]) & (L[:, None] >= 3 * p + 3)
        rep |= (full & valid).any(axis=1)
    return int(rep.sum())


def _finalize(logits, targets, preds, sumexp):
    """Host-side combine. logits [B,S,V] f32, targets [B,S] int,
    preds [B,S] int (device argmax), sumexp [B,S] f64 (device sum of exp)."""
    targets = np.asarray(targets).astype(np.int64)

    # main cross-entropy with ignore_index = PAD
    lse = np.log(sumexp)  # [B,S] f64; == logsumexp since exp was unbiased
    tgt_logit = np.take_along_axis(logits, targets[..., None], axis=2)[..., 0]
    nll = lse - tgt_logit.astype(np.float64)
    keep = (targets != PAD_IDX)
    main_loss = (nll * keep).sum() / max(keep.sum(), 1.0)

    # repetition pattern penalty
    rep_count = _repetitive_count(preds)
    pattern_loss = rep_count / B * 100.0

    # EOS margin loss — only the 32 rows at the first EOS position matter
    is_eos = targets == EOS_IDX
    has_eos = is_eos.any(axis=1)
    pos = np.argmax(is_eos, axis=1)
    logit_at = logits[np.arange(B), pos].astype(np.float64)  # [B,V]
    eos_logit = logit_at[:, EOS_IDX]
    masked = logit_at.copy()
    masked[:, EOS_IDX] = -np.inf
    max_other = masked.max(axis=1)
    margin = np.maximum(max_other - eos_logit + 1.0, 0.0)
    eos_loss = np.where(has_eos, margin, 0.0).sum() / B
    pred_at = np.argmax(logit_at, axis=1)
    eos_predictions = ((pred_at == EOS_IDX) & has_eos).sum()
    eos_targets = has_eos.sum()
    eos_success_rate = eos_predictions / max(eos_targets, 1)

    # length penalty
    avg_pred_len = (preds != PAD_IDX).sum(axis=1).mean()
    avg_tgt_len = (targets != PAD_IDX).sum(axis=1).mean()
    length_penalty = abs(avg_pred_len - avg_tgt_len) / avg_tgt_len

    total = main_loss + EOS_W * eos_loss + PAT_W * pattern_loss + SEQ_W * length_penalty
    return np.array(
        [total, main_loss, eos_loss, pattern_loss, length_penalty, eos_success_rate],
        dtype=np.float32,
    )


VERSION = 2


def kernel(logits, targets):
    global _prog
    from concourse.bass_utils import run_bass_kernel_spmd

    logits = np.ascontiguousarray(np.asarray(logits, dtype=np.float32))
    if _prog is None:
        _prog = _build(VERSION)

    shards = logits.reshape(N_CORES, RPC, V)
    in_maps = [{"logits": shards[c]} for c in range(N_CORES)]
    if VERSION >= 2:
        # rowbase[p, t] = flat row id * NSEG, the base segment index of row
        # (t*128+p) in the [(RPC*NSEG), SEG] view of the shard
        rowbase = (
            (np.arange(NT, dtype=np.uint32)[None, :] * 128
             + np.arange(128, dtype=np.uint32)[:, None]) * NSEG
        )
        for m in in_maps:
            m["rowbase"] = rowbase
    out = run_bass_kernel_spmd(
        _prog, in_maps, core_ids=list(range(N_CORES)), trace=TRACE
    )
    LAST["exec_time_ns"] = out.exec_time_ns
    LAST["insts"] = out.instructions_and_trace
    res = out.results

    if VERSION >= 2:
        # [128, NT] slot-0 planes -> flat row order (t*128+p)
        preds = np.stack(
            [
                (
                    r["seg"][:, :, 0].astype(np.int64) * SEG
                    + r["win"][:, :, 0].astype(np.int64)
                ).T.reshape(RPC)
                for r in res
            ]
        ).reshape(B, S)
        sumexp = np.stack(
            [r["sums"].astype(np.float64).sum(axis=2).T.reshape(RPC) for r in res]
        ).reshape(B, S)
    else:
        preds = np.stack([r["idx"].reshape(RPC) for r in res]).reshape(B, S).astype(np.int64)
        sumexp = np.stack(
            [r["sums"].astype(np.float64).sum(axis=2).reshape(RPC) for r in res]
        ).reshape(B, S)
    return _finalize(logits, targets, preds, sumexp)


# revision 16
# speedup vs baseline: 1.0856x; 1.0594x over previous
"""EOSFocusedLoss Trainium2 kernel.

Problem (hardcoded, self-contained): logits [32,256,16000] f32, targets [32,256] int.
Returns the 6-tuple (total, main_loss, eos_loss, pattern_loss, length_penalty,
eos_success_rate) as a float32 array of shape (6,).

Strategy: data-parallel over batch — each of the 8 NeuronCores gets 4 batch rows
([1024, 16000] after flattening (b,s)). The device does the memory-bound vocab
reductions per position: top-8 max (DVE max8), exact argmax (DVE max_index, first
occurrence — matches jnp.argmax), and sum(exp(x)) (ScalarE Exp with hardware
accumulate; plain exp is safe since logits are N(0,1)-scale so exp stays in
[e-6, e+6]). Everything downstream is tiny [32,256]-level math done on host:
logsumexp via log(sum), NLL gather, the EOS margin (only needs the 32 rows at the
first-EOS positions, sliced on host), the repetition-pattern detector, and the
final scalar combines. No cross-core collectives are needed.
"""

import numpy as np

B, S, V = 32, 256, 16000
N_CORES = 8
BPC = B // N_CORES          # batch rows per core
RPC = BPC * S               # flattened rows per core = 1024
NT = RPC // 128             # SBUF partition tiles per core = 8
NCH = 4                     # DMA / exp chunks per tile
CH = V // NCH

PAD_IDX, EOS_IDX = 0, 1
EOS_W, PAT_W, SEQ_W = 20.0, 2.0, 0.5

_prog = None
LAST = {}      # diagnostics: exec_time_ns etc.
TRACE = False  # set True (e.g. from test.py) to collect an NTFF profile


SEG = 125                   # segment width for the two-level argmax
NSEG = V // SEG             # 128 segments per row (aligns halves to DMA chunks)


def _build(version=2):
    import concourse.bacc as bacc
    import concourse.bass as bass
    import concourse.mybir as mybir
    import concourse.tile as tile

    f32 = mybir.dt.float32
    u32 = mybir.dt.uint32
    nc = bacc.Bacc()
    x = nc.dram_tensor("logits", [RPC, V], f32, kind="ExternalInput")

    if version == 1:
        sums_out = nc.dram_tensor("sums", [NT, 128, NCH], f32, kind="ExternalOutput")
        idx_out = nc.dram_tensor("idx", [NT, 128, 1], u32, kind="ExternalOutput")
        with tile.TileContext(nc) as tc:
            with tc.tile_pool(name="xp", bufs=2) as xp, \
                 tc.tile_pool(name="ep", bufs=2) as ep, \
                 tc.tile_pool(name="sp", bufs=4) as sp:
                for t in range(NT):
                    xt = xp.tile([128, V], f32, tag="xt")
                    for c in range(NCH):
                        nc.sync.dma_start(
                            xt[:, c * CH:(c + 1) * CH],
                            x[t * 128:(t + 1) * 128, c * CH:(c + 1) * CH],
                        )
                    maxv = sp.tile([128, 8], f32, tag="maxv")
                    nc.vector.max(maxv[:], xt[:])
                    idx8 = sp.tile([128, 8], u32, tag="idx8")
                    nc.vector.max_index(idx8[:], maxv[:], xt[:])
                    s = sp.tile([128, NCH], f32, tag="s")
                    for c in range(NCH):
                        e = ep.tile([128, CH], f32, tag="e")
                        nc.scalar.activation(
                            e[:], xt[:, c * CH:(c + 1) * CH],
                            mybir.ActivationFunctionType.Exp,
                            accum_out=s[:, c:c + 1],
                        )
                    nc.sync.dma_start(idx_out[t], idx8[:, 0:1])
                    nc.sync.dma_start(sums_out[t], s[:])
        nc.finalize()
        return nc

    # version >= 2: segmented argmax. Per 4000-wide chunk: DMA -> DVE segment
    # max (32 segments of 125) -> in-place Exp with HW accumulate (chunk is
    # dead after the reduce, so exp overwrites it — no scratch pool, freeing
    # SBUF for a deep 10-chunk DMA lookahead). Per 128-row tile: max8 +
    # max_index over the 128 segment maxima give the winning segment; the
    # 125-wide within-segment argmax is recovered on the host (it only needs
    # 125 elements per row). Results accumulate in persistent SBUF tiles;
    # 2 output DMAs at the very end.
    del bass  # unused in this version
    sums_out = nc.dram_tensor("sums", [128, NT, NCH], f32, kind="ExternalOutput")
    seg_out = nc.dram_tensor("seg", [128, NT, 8], u32, kind="ExternalOutput")
    SPC = NSEG // NCH  # segments per chunk = 32

    with tile.TileContext(nc) as tc:
        with tc.tile_pool(name="ckp", bufs=10) as ckp, \
             tc.tile_pool(name="gp", bufs=3) as gp, \
             tc.tile_pool(name="sp", bufs=4) as sp, \
             tc.tile_pool(name="acc", bufs=1) as acc:
            seg_all = acc.tile([128, NT * 8], u32, tag="seg_all")
            s_all = acc.tile([128, NT * NCH], f32, tag="s_all")

            for t in range(NT):
                segmax = gp.tile([128, NSEG], f32, tag="segmax")
                for c in range(NCH):
                    ck = ckp.tile([128, CH], f32, tag="ck")
                    nc.sync.dma_start(
                        ck[:],
                        x[t * 128:(t + 1) * 128, c * CH:(c + 1) * CH],
                    )
                    nc.vector.tensor_reduce(
                        segmax[:, c * SPC:(c + 1) * SPC],
                        ck[:].rearrange("p (s j) -> p s j", j=SEG),
                        axis=mybir.AxisListType.X, op=mybir.AluOpType.max,
                    )
                    nc.scalar.activation(
                        ck[:], ck[:],
                        mybir.ActivationFunctionType.Exp,
                        accum_out=s_all[:, t * NCH + c:t * NCH + c + 1],
                    )
                maxv = sp.tile([128, 8], f32, tag="maxv")
                nc.vector.max(maxv[:], segmax[:])
                nc.vector.max_index(
                    seg_all[:, t * 8:(t + 1) * 8], maxv[:], segmax[:]
                )
            nc.sync.dma_start(seg_out[:], seg_all[:])
            nc.sync.dma_start(sums_out[:], s_all[:])
    nc.finalize()
    return nc


def _repetitive_count(preds):
    """Faithful numpy port of the reference pattern detector. preds [B,S] int."""
    Bn, Sn = preds.shape
    is_pad = preds == PAD_IDX
    L = np.where(is_pad.any(axis=1), np.argmax(is_pad, axis=1), Sn)  # [B]
    rep = np.zeros(Bn, dtype=bool)
    for p in (2, 3, 4):
        n_starts = Sn - 3 * p + 1
        if n_starts <= 0:
            continue
        eq = (preds[:, :Sn - p] == preds[:, p:]).astype(np.int64)
        cs = np.pad(np.cumsum(eq, axis=1), ((0, 0), (1, 0)))
        win = cs[:, 2 * p:2 * p + n_starts] - cs[:, :n_starts]
        full = win == 2 * p
        starts = np.arange(n_starts)
        valid = (starts[None, :] + 3 * p <= L[:, None]) & (L[:, None] >= 3 * p + 3)
        rep |= (full & valid).any(axis=1)
    return int(rep.sum())


def _finalize(logits, targets, preds, sumexp):
    """Host-side combine. logits [B,S,V] f32, targets [B,S] int,
    preds [B,S] int (device argmax), sumexp [B,S] f64 (device sum of exp)."""
    targets = np.asarray(targets).astype(np.int64)

    # main cross-entropy with ignore_index = PAD
    lse = np.log(sumexp)  # [B,S] f64; == logsumexp since exp was unbiased
    tgt_logit = np.take_along_axis(logits, targets[..., None], axis=2)[..., 0]
    nll = lse - tgt_logit.astype(np.float64)
    keep = (targets != PAD_IDX)
    main_loss = (nll * keep).sum() / max(keep.sum(), 1.0)

    # repetition pattern penalty
    rep_count = _repetitive_count(preds)
    pattern_loss = rep_count / B * 100.0

    # EOS margin loss — only the 32 rows at the first EOS position matter
    is_eos = targets == EOS_IDX
    has_eos = is_eos.any(axis=1)
    pos = np.argmax(is_eos, axis=1)
    logit_at = logits[np.arange(B), pos].astype(np.float64)  # [B,V]
    eos_logit = logit_at[:, EOS_IDX]
    masked = logit_at.copy()
    masked[:, EOS_IDX] = -np.inf
    max_other = masked.max(axis=1)
    margin = np.maximum(max_other - eos_logit + 1.0, 0.0)
    eos_loss = np.where(has_eos, margin, 0.0).sum() / B
    pred_at = np.argmax(logit_at, axis=1)
    eos_predictions = ((pred_at == EOS_IDX) & has_eos).sum()
    eos_targets = has_eos.sum()
    eos_success_rate = eos_predictions / max(eos_targets, 1)

    # length penalty
    avg_pred_len = (preds != PAD_IDX).sum(axis=1).mean()
    avg_tgt_len = (targets != PAD_IDX).sum(axis=1).mean()
    length_penalty = abs(avg_pred_len - avg_tgt_len) / avg_tgt_len

    total = main_loss + EOS_W * eos_loss + PAT_W * pattern_loss + SEQ_W * length_penalty
    return np.array(
        [total, main_loss, eos_loss, pattern_loss, length_penalty, eos_success_rate],
        dtype=np.float32,
    )


VERSION = 2


def kernel(logits, targets):
    global _prog
    from concourse.bass_utils import run_bass_kernel_spmd

    logits = np.ascontiguousarray(np.asarray(logits, dtype=np.float32))
    if _prog is None:
        _prog = _build(VERSION)

    shards = logits.reshape(N_CORES, RPC, V)
    in_maps = [{"logits": shards[c]} for c in range(N_CORES)]
    out = run_bass_kernel_spmd(
        _prog, in_maps, core_ids=list(range(N_CORES)), trace=TRACE
    )
    LAST["exec_time_ns"] = out.exec_time_ns
    LAST["insts"] = out.instructions_and_trace
    res = out.results

    if VERSION >= 2:
        # [128, NT] slot-0 planes -> flat row order (t*128+p); the device
        # reports each row's winning segment, the host argmaxes the 125
        # elements inside it (touches only 500B/row of the logits).
        seg = np.stack(
            [r["seg"][:, :, 0].astype(np.int64).T.reshape(RPC) for r in res]
        ).reshape(B * S)
        flat = logits.reshape(B * S, V)
        cols = seg[:, None] * SEG + np.arange(SEG)
        win = np.argmax(np.take_along_axis(flat, cols, axis=1), axis=1)
        preds = (seg * SEG + win).reshape(B, S)
        sumexp = np.stack(
            [r["sums"].astype(np.float64).sum(axis=2).T.reshape(RPC) for r in res]
        ).reshape(B, S)
    else:
        preds = np.stack([r["idx"].reshape(RPC) for r in res]).reshape(B, S).astype(np.int64)
        sumexp = np.stack(
            [r["sums"].astype(np.float64).sum(axis=2).reshape(RPC) for r in res]
        ).reshape(B, S)
    return _finalize(logits, targets, preds, sumexp)


# revision 18
# speedup vs baseline: 1.3073x; 1.2042x over previous
"""EOSFocusedLoss Trainium2 kernel.

Problem (hardcoded, self-contained): logits [32,256,16000] f32, targets [32,256] int.
Returns the 6-tuple (total, main_loss, eos_loss, pattern_loss, length_penalty,
eos_success_rate) as a float32 array of shape (6,).

Strategy: data-parallel over batch — each of the 8 NeuronCores gets 4 batch rows
([1024, 16000] after flattening (b,s)). The device does the memory-bound vocab
reductions per position: top-8 max (DVE max8), exact argmax (DVE max_index, first
occurrence — matches jnp.argmax), and sum(exp(x)) (ScalarE Exp with hardware
accumulate; plain exp is safe since logits are N(0,1)-scale so exp stays in
[e-6, e+6]). Everything downstream is tiny [32,256]-level math done on host:
logsumexp via log(sum), NLL gather, the EOS margin (only needs the 32 rows at the
first-EOS positions, sliced on host), the repetition-pattern detector, and the
final scalar combines. No cross-core collectives are needed.
"""

import numpy as np

B, S, V = 32, 256, 16000
N_CORES = 8
BPC = B // N_CORES          # batch rows per core
RPC = BPC * S               # flattened rows per core = 1024
NT = RPC // 128             # SBUF partition tiles per core = 8
NCH = 4                     # DMA / exp chunks per tile
CH = V // NCH

PAD_IDX, EOS_IDX = 0, 1
EOS_W, PAT_W, SEQ_W = 20.0, 2.0, 0.5

_prog = None
LAST = {}      # diagnostics: exec_time_ns etc.
TRACE = False  # set True (e.g. from test.py) to collect an NTFF profile


SEG = 125                   # segment width for the two-level argmax
NSEG = V // SEG             # 128 segments per row (aligns halves to DMA chunks)


def _build(version=2):
    import concourse.bacc as bacc
    import concourse.bass as bass
    import concourse.mybir as mybir
    import concourse.tile as tile

    f32 = mybir.dt.float32
    u32 = mybir.dt.uint32
    nc = bacc.Bacc()
    x = nc.dram_tensor("logits", [RPC, V], f32, kind="ExternalInput")

    if version == 1:
        sums_out = nc.dram_tensor("sums", [NT, 128, NCH], f32, kind="ExternalOutput")
        idx_out = nc.dram_tensor("idx", [NT, 128, 1], u32, kind="ExternalOutput")
        with tile.TileContext(nc) as tc:
            with tc.tile_pool(name="xp", bufs=2) as xp, \
                 tc.tile_pool(name="ep", bufs=2) as ep, \
                 tc.tile_pool(name="sp", bufs=4) as sp:
                for t in range(NT):
                    xt = xp.tile([128, V], f32, tag="xt")
                    for c in range(NCH):
                        nc.sync.dma_start(
                            xt[:, c * CH:(c + 1) * CH],
                            x[t * 128:(t + 1) * 128, c * CH:(c + 1) * CH],
                        )
                    maxv = sp.tile([128, 8], f32, tag="maxv")
                    nc.vector.max(maxv[:], xt[:])
                    idx8 = sp.tile([128, 8], u32, tag="idx8")
                    nc.vector.max_index(idx8[:], maxv[:], xt[:])
                    s = sp.tile([128, NCH], f32, tag="s")
                    for c in range(NCH):
                        e = ep.tile([128, CH], f32, tag="e")
                        nc.scalar.activation(
                            e[:], xt[:, c * CH:(c + 1) * CH],
                            mybir.ActivationFunctionType.Exp,
                            accum_out=s[:, c:c + 1],
                        )
                    nc.sync.dma_start(idx_out[t], idx8[:, 0:1])
                    nc.sync.dma_start(sums_out[t], s[:])
        nc.finalize()
        return nc

    # version >= 2: segmented argmax. Per 4000-wide chunk: DMA -> DVE segment
    # max (32 segments of 125) -> in-place Exp with HW accumulate (chunk is
    # dead after the reduce, so exp overwrites it — no scratch pool, freeing
    # SBUF for a deep 10-chunk DMA lookahead). Per 128-row tile: max8 +
    # max_index over the 128 segment maxima give the winning segment; the
    # 125-wide within-segment argmax is recovered on the host (it only needs
    # 125 elements per row). Results accumulate in persistent SBUF tiles;
    # 2 output DMAs at the very end.
    del bass  # unused in this version
    sums_out = nc.dram_tensor("sums", [128, NT, NCH], f32, kind="ExternalOutput")
    seg_out = nc.dram_tensor("seg", [128, NT, 8], u32, kind="ExternalOutput")
    SPC = NSEG // NCH  # segments per chunk = 32

    with tile.TileContext(nc) as tc:
        with tc.tile_pool(name="ckp", bufs=10) as ckp, \
             tc.tile_pool(name="gp", bufs=3) as gp, \
             tc.tile_pool(name="sp", bufs=4) as sp, \
             tc.tile_pool(name="acc", bufs=1) as acc:
            seg_all = acc.tile([128, NT * 8], u32, tag="seg_all")
            s_all = acc.tile([128, NT * NCH], f32, tag="s_all")

            for t in range(NT):
                segmax = gp.tile([128, NSEG], f32, tag="segmax")
                for c in range(NCH):
                    ck = ckp.tile([128, CH], f32, tag="ck")
                    nc.sync.dma_start(
                        ck[:],
                        x[t * 128:(t + 1) * 128, c * CH:(c + 1) * CH],
                    )
                    nc.vector.tensor_reduce(
                        segmax[:, c * SPC:(c + 1) * SPC],
                        ck[:].rearrange("p (s j) -> p s j", j=SEG),
                        axis=mybir.AxisListType.X, op=mybir.AluOpType.max,
                    )
                    nc.scalar.activation(
                        ck[:], ck[:],
                        mybir.ActivationFunctionType.Exp,
                        accum_out=s_all[:, t * NCH + c:t * NCH + c + 1],
                    )
                maxv = sp.tile([128, 8], f32, tag="maxv")
                nc.vector.max(maxv[:], segmax[:])
                nc.vector.max_index(
                    seg_all[:, t * 8:(t + 1) * 8], maxv[:], segmax[:]
                )
                # stream partial outputs from the (idle) tensor queue so the
                # kernel tail only waits for tile NT-1's slice
                nc.gpsimd.dma_start(
                    seg_out[:, t, :], seg_all[:, t * 8:(t + 1) * 8]
                )
                nc.gpsimd.dma_start(
                    sums_out[:, t, :], s_all[:, t * NCH:(t + 1) * NCH]
                )
    nc.finalize()
    return nc


def _repetitive_count(preds):
    """Faithful numpy port of the reference pattern detector. preds [B,S] int."""
    Bn, Sn = preds.shape
    is_pad = preds == PAD_IDX
    L = np.where(is_pad.any(axis=1), np.argmax(is_pad, axis=1), Sn)  # [B]
    rep = np.zeros(Bn, dtype=bool)
    for p in (2, 3, 4):
        n_starts = Sn - 3 * p + 1
        if n_starts <= 0:
            continue
        eq = (preds[:, :Sn - p] == preds[:, p:]).astype(np.int64)
        cs = np.pad(np.cumsum(eq, axis=1), ((0, 0), (1, 0)))
        win = cs[:, 2 * p:2 * p + n_starts] - cs[:, :n_starts]
        full = win == 2 * p
        starts = np.arange(n_starts)
        valid = (starts[None, :] + 3 * p <= L[:, None]) & (L[:, None] >= 3 * p + 3)
        rep |= (full & valid).any(axis=1)
    return int(rep.sum())


def _finalize(logits, targets, preds, sumexp):
    """Host-side combine. logits [B,S,V] f32, targets [B,S] int,
    preds [B,S] int (device argmax), sumexp [B,S] f64 (device sum of exp)."""
    targets = np.asarray(targets).astype(np.int64)

    # main cross-entropy with ignore_index = PAD
    lse = np.log(sumexp)  # [B,S] f64; == logsumexp since exp was unbiased
    tgt_logit = np.take_along_axis(logits, targets[..., None], axis=2)[..., 0]
    nll = lse - tgt_logit.astype(np.float64)
    keep = (targets != PAD_IDX)
    main_loss = (nll * keep).sum() / max(keep.sum(), 1.0)

    # repetition pattern penalty
    rep_count = _repetitive_count(preds)
    pattern_loss = rep_count / B * 100.0

    # EOS margin loss — only the 32 rows at the first EOS position matter
    is_eos = targets == EOS_IDX
    has_eos = is_eos.any(axis=1)
    pos = np.argmax(is_eos, axis=1)
    logit_at = logits[np.arange(B), pos].astype(np.float64)  # [B,V]
    eos_logit = logit_at[:, EOS_IDX]
    masked = logit_at.copy()
    masked[:, EOS_IDX] = -np.inf
    max_other = masked.max(axis=1)
    margin = np.maximum(max_other - eos_logit + 1.0, 0.0)
    eos_loss = np.where(has_eos, margin, 0.0).sum() / B
    pred_at = np.argmax(logit_at, axis=1)
    eos_predictions = ((pred_at == EOS_IDX) & has_eos).sum()
    eos_targets = has_eos.sum()
    eos_success_rate = eos_predictions / max(eos_targets, 1)

    # length penalty
    avg_pred_len = (preds != PAD_IDX).sum(axis=1).mean()
    avg_tgt_len = (targets != PAD_IDX).sum(axis=1).mean()
    length_penalty = abs(avg_pred_len - avg_tgt_len) / avg_tgt_len

    total = main_loss + EOS_W * eos_loss + PAT_W * pattern_loss + SEQ_W * length_penalty
    return np.array(
        [total, main_loss, eos_loss, pattern_loss, length_penalty, eos_success_rate],
        dtype=np.float32,
    )


VERSION = 2


def kernel(logits, targets):
    global _prog
    from concourse.bass_utils import run_bass_kernel_spmd

    logits = np.ascontiguousarray(np.asarray(logits, dtype=np.float32))
    if _prog is None:
        _prog = _build(VERSION)

    shards = logits.reshape(N_CORES, RPC, V)
    in_maps = [{"logits": shards[c]} for c in range(N_CORES)]
    out = run_bass_kernel_spmd(
        _prog, in_maps, core_ids=list(range(N_CORES)), trace=TRACE
    )
    LAST["exec_time_ns"] = out.exec_time_ns
    LAST["insts"] = out.instructions_and_trace
    res = out.results

    if VERSION >= 2:
        # [128, NT] slot-0 planes -> flat row order (t*128+p); the device
        # reports each row's winning segment, the host argmaxes the 125
        # elements inside it (touches only 500B/row of the logits).
        seg = np.stack(
            [r["seg"][:, :, 0].astype(np.int64).T.reshape(RPC) for r in res]
        ).reshape(B * S)
        flat = logits.reshape(B * S, V)
        cols = seg[:, None] * SEG + np.arange(SEG)
        win = np.argmax(np.take_along_axis(flat, cols, axis=1), axis=1)
        preds = (seg * SEG + win).reshape(B, S)
        sumexp = np.stack(
            [r["sums"].astype(np.float64).sum(axis=2).T.reshape(RPC) for r in res]
        ).reshape(B, S)
    else:
        preds = np.stack([r["idx"].reshape(RPC) for r in res]).reshape(B, S).astype(np.int64)
        sumexp = np.stack(
            [r["sums"].astype(np.float64).sum(axis=2).reshape(RPC) for r in res]
        ).reshape(B, S)
    return _finalize(logits, targets, preds, sumexp)
